# revision 2
# baseline (speedup 1.0000x reference)
"""BinaryLinear (straight-through sign(w)) kernel for Trainium2, 8 NeuronCores.

Computes out = x @ sign(w).T + b for
  x: [8192, 2048] f32, w: [4096, 2048] f32, b: [4096] f32 -> out [8192, 4096] f32.

Sharding: 4-way data parallel (batch) x 2-way tensor parallel (out_features).
Each core computes a [2048, 2048] block of the output:
  out[bi*2048:(bi+1)*2048, fi*2048:(fi+1)*2048]
    = x_shard @ sign(w_shard).T + b_shard.

Per-core device kernel (fp16 matmul, fp32 accumulate — fp16 runs at the same
PE rate as bf16 but keeps 10 mantissa bits; sign(w) in {-1,0,1} is exact):
  - the whole w^T shard [2048, 2048] fp16 lives in SBUF (64 KiB/partition),
    loaded once;
  - x^T tiles stream through a multi-buffered pool;
  - bias is added during the PSUM->SBUF copyback on the vector engine.
"""

from contextlib import ExitStack

import numpy as np

# Full problem shapes (hardcoded per the grading contract).
M, K, N = 8192, 2048, 4096
P_BATCH, P_FEAT = 4, 2  # 4 x 2 core grid
MC, NC = M // P_BATCH, N // P_FEAT  # 2048, 2048 per-core block
N_CORES = P_BATCH * P_FEAT
P = 128


def build_nc(mc: int = MC, k: int = K, nc_dim: int = NC, reps: int = 1):
    """Build + compile the per-core Bass module: out[mc, nc_dim] = xt^T @ wt + bias.

    reps > 1 repeats the whole computation (for slope-based benchmarking)."""
    import concourse.mybir as mybir
    import concourse.tile as tile
    from concourse import bacc
    from concourse.bass import ts
    from concourse.kernels.tile_matmul import (
        ShapeInfo,
        composable_matmul_tile_kernel,
    )

    ko = k // P
    MAX_K_TILE = 512
    k_tile = min(MAX_K_TILE, k)
    k_tiles = k // k_tile
    k_subtiles = k_tile // P
    TB = 512  # m/n tile width of the pre-blocked host layouts
    m_tiles = mc // TB
    n_blocks = nc_dim // TB

    nc = bacc.Bacc("TRN2", target_bir_lowering=False, debug=False)
    # Inputs arrive pre-blocked on the host (see _pack_blocks): each
    # [P, k_subtiles, TB] block is fully contiguous in DRAM, so every DMA has
    # 4-KiB-per-partition descriptor runs instead of 1-KiB strided ones.
    xt = nc.dram_tensor(
        "xt", [m_tiles, k_tiles, P, k_subtiles, TB], mybir.dt.float16,
        kind="ExternalInput",
    )
    # w uses an n-block-major layout ([nb, p, ko, n]) so each n-block's
    # preload is contiguous per partition on BOTH sides: 16-KiB descriptor
    # runs instead of 4-KiB, 4x fewer descriptors for the Q7 to emit.
    wt = nc.dram_tensor(
        "wt", [n_blocks, P, ko, TB], mybir.dt.float16, kind="ExternalInput"
    )
    bias = nc.dram_tensor("bias", [nc_dim], mybir.dt.float32, kind="ExternalInput")
    out = nc.dram_tensor("out", [mc, nc_dim], mybir.dt.float32, kind="ExternalOutput")

    with tile.TileContext(nc) as tc, ExitStack() as ctx:
        # HAM warmup: the PE clock is gated to 1.2 GHz until ~3.4 us of
        # sustained activity. The first real matmuls can't start until their
        # operands arrive (~5 us of DMA ramp), so spend the idle window on
        # throwaway matmuls over a zeroed scratch tile — the cold-clock
        # penalty lands on them instead of the real work. The scratch SBUF
        # pool stays OPEN so its slot is never reused (a close would order
        # the w preload behind the dummy reads); only the PSUM bank is
        # returned before the real kernel needs all 8.
        warm_sb = ctx.enter_context(tc.tile_pool(name="warm_sb", bufs=1))
        scratch = warm_sb.tile([P, 512], mybir.dt.float16)
        nc.vector.memset(scratch[:], 0.0)
        with tc.tile_pool(name="warm_ps", bufs=1, space="PSUM") as wps_pool:
            ps = wps_pool.tile([P, 512], mybir.dt.float32)
            for _ in range(10):
                nc.tensor.matmul(
                    ps[:], scratch[:, :P], scratch[:], start=True, stop=True
                )

        const = ctx.enter_context(tc.tile_pool(name="const", bufs=1))
        kxm_pool = ctx.enter_context(tc.tile_pool(name="kxm", bufs=k_tiles + 1))

        # Whole w^T shard resident in SBUF, n-block-major [p, nb, ko, n] with
        # cache[p, nb, o, j] = w^T[o*128 + p, nb*TB + j]. Preload runs on the
        # gpsimd (SWDGE) queue so the x-tile loads (HWDGE via nc.sync) are not
        # serialized behind it, in n-major order: the first output tile
        # consumes (n0, k0..k3), so all its chunks must land first.
        w_sb = const.tile([P, n_blocks, ko, TB], mybir.dt.float16)
        for nb in range(n_blocks):
            if nb == 0:
                # first block split per k-tile so the first matmuls unblock
                # at k-tile granularity
                for kt in range(k_tiles):
                    sl = slice(kt * k_subtiles, (kt + 1) * k_subtiles)
                    nc.gpsimd.dma_start(
                        out=w_sb[:, 0, sl, :], in_=wt.ap()[0, :, sl, :]
                    )
            else:
                nc.gpsimd.dma_start(out=w_sb[:, nb], in_=wt.ap()[nb])

        # Bias replicated across all 128 partitions so the copyback can add the
        # n-slice with a plain tensor_tensor add. One tiny [1, N] HBM read on
        # the otherwise-idle ACT HWDGE ring + an on-chip partition broadcast —
        # a [128, N] broadcast DMA on the SWDGE queue behind the w preload
        # would block the first evictions (and PSUM recycling) until ~36 us.
        bias_sb = const.tile([P, nc_dim], mybir.dt.float32)
        nc.scalar.dma_start(out=bias_sb[:1, :], in_=bias.ap()[None, :])
        nc.gpsimd.partition_broadcast(bias_sb[:], bias_sb[:1, :])

        # Custom kxm producer: one contiguous-block DMA per k-tile of x^T.
        def kxm_producer(nc_, md):
            t = kxm_pool.tile([P, md.k_subtiles, md.m_tile], mybir.dt.float16, tag="kxm")
            if md.m_tile_idx == 1 and n_blocks > 2:
                # Ordering-only dep: m1's prefetch is not needed until ~60 us
                # but otherwise jumps the shared DMA mover ahead of the w n1/n2
                # blocks (needed at ~20/~33 us). A tiny read of the w n2 region
                # into this tile makes the real load schedule after that w
                # block has transferred.
                nc_.vector.tensor_copy(out=t[:1, :1, :2], in_=w_sb[:1, 2, :1, :2])
            nc_.sync.dma_start(out=t[:], in_=xt.ap()[md.m_tile_idx, md.k_tile_idx])
            return t

        kxm_shape = ShapeInfo(pdims=((P, ko),), fdims=(mc,))

        def kxn_producer(nc_, md):
            return w_sb[:, md.n_tile_idx, ts(md.k_tile_idx, md.k_subtiles), :]

        kxn_shape = ShapeInfo(pdims=((P, ko),), fdims=(nc_dim,))

        out_t = out.ap().rearrange("(o p) n -> p o n", p=P)

        def add_bias_store_reducer(nc_, psum, sbuf, md):
            # psum -> sbuf with the bias added, then store this subtile
            # immediately (finer-grained than the stock whole-tile consumer,
            # so stores overlap the remaining evictions and the tail drains
            # faster).
            sz = md.n_subtile_slice_size
            nc_.vector.tensor_add(
                out=sbuf[:, :, :sz],
                in0=psum[:, :sz],
                in1=bias_sb[: psum.shape[0], md.n_subtile_slice],
            )
            po = md.m_tile_idx * md.m_subtiles + md.m_subtile_idx
            nc_.sync.dma_start(
                out=out_t[:, po : po + 1, md.n_subtile_slice], in_=sbuf[:, :, :sz]
            )

        for _ in range(reps):
            composable_matmul_tile_kernel(
                tc=tc,
                kxm_shape=kxm_shape,
                kxn_shape=kxn_shape,
                output_type=mybir.dt.float32,
                kxm_producer=kxm_producer,
                kxn_producer=kxn_producer,
                mxn_consumer=lambda nc_, tile_, md: None,
                mxn_subtile_reducer=add_bias_store_reducer,
                MAX_K_TILE_SIZE=MAX_K_TILE,
                psum_n_bufs=2,
            )

    nc.compile()
    return nc


def _pack_w_nblocks(a: np.ndarray, tb: int = 512) -> np.ndarray:
    """[N, K] row-major -> [N//tb, 128, K//128, tb] with
    block[nb, p, o, j] = a[nb*tb + j, o*128 + p]; per-partition-contiguous
    [ko, tb] planes -> 16-KiB DMA descriptor runs."""
    n, k = a.shape
    v = a.reshape(n // tb, tb, k // P, P)
    return np.ascontiguousarray(v.transpose(0, 3, 2, 1))


def _pack_blocks(a: np.ndarray, tb: int = 512) -> np.ndarray:
    """[F, K] row-major -> [F//tb, K//ktw, 128, ks, tb] DMA-contiguous blocks.

    block[ft, kt, p, s, j] = a[ft*tb + j, kt*ktw + s*128 + p], i.e. each
    [128, ks, tb] block is one fully-contiguous DMA source with K on the
    partition dim (a^T layout within the block)."""
    f, k = a.shape
    ktw = min(512, k)
    kts, ks = k // ktw, ktw // P
    v = a.reshape(f // tb, tb, kts, ks, P)
    return np.ascontiguousarray(v.transpose(0, 2, 4, 3, 1))


_NC_CACHE = None


def _get_nc():
    global _NC_CACHE
    if _NC_CACHE is None:
        _NC_CACHE = build_nc()
    return _NC_CACHE


def build_in_maps(x: np.ndarray, w: np.ndarray, b: np.ndarray) -> list[dict]:
    f16 = np.float16
    x_f16 = x.astype(f16)
    w_f16 = np.sign(w).astype(f16)

    # Unique DMA-blocked shards (x per batch group, sign(w) per feature
    # group), packed in parallel (numpy releases the GIL on these copies).
    from concurrent.futures import ThreadPoolExecutor

    with ThreadPoolExecutor(max_workers=6) as pool:
        xt_f = [
            pool.submit(_pack_blocks, x_f16[bi * MC : (bi + 1) * MC])
            for bi in range(P_BATCH)
        ]
        wt_f = [
            pool.submit(_pack_w_nblocks, w_f16[fi * NC : (fi + 1) * NC])
            for fi in range(P_FEAT)
        ]
        xt_shards = [f.result() for f in xt_f]
        wt_shards = [f.result() for f in wt_f]
    b_shards = [np.ascontiguousarray(b[fi * NC : (fi + 1) * NC]) for fi in range(P_FEAT)]

    in_maps = []
    for c in range(N_CORES):
        bi, fi = divmod(c, P_FEAT)
        in_maps.append(
            {"xt": xt_shards[bi], "wt": wt_shards[fi], "bias": b_shards[fi]}
        )
    return in_maps


def kernel(x: np.ndarray, w: np.ndarray, b: np.ndarray) -> np.ndarray:
    from concourse.bass_utils import run_bass_kernel_spmd

    x = np.asarray(x, dtype=np.float32)
    w = np.asarray(w, dtype=np.float32)
    b = np.asarray(b, dtype=np.float32)

    in_maps = build_in_maps(x, w, b)

    nc = _get_nc()
    try:
        results = run_bass_kernel_spmd(
            nc, in_maps, core_ids=list(range(N_CORES))
        ).results
    except Exception:
        # One retry for transient runtime/relay failures.
        results = run_bass_kernel_spmd(
            nc, in_maps, core_ids=list(range(N_CORES))
        ).results

    out = np.empty((M, N), dtype=np.float32)
    for c in range(N_CORES):
        bi, fi = divmod(c, P_FEAT)
        out[bi * MC : (bi + 1) * MC, fi * NC : (fi + 1) * NC] = results[c]["out"]
    return out



# revision 3
# speedup vs baseline: 2.0022x; 2.0022x over previous
"""BinaryLinear (straight-through sign(w)) kernel for Trainium2, 8 NeuronCores.

Computes out = x @ sign(w).T + b for
  x: [8192, 2048] f32, w: [4096, 2048] f32, b: [4096] f32 -> out [8192, 4096] f32.

Sharding: 4-way data parallel (batch) x 2-way tensor parallel (out_features).
Each core computes a [2048, 2048] block of the output.

Per-core device kernel — fp8e4 (e4m3) matmuls in DoubleRow perf mode (0.5
PE cycles per output row, 2x-4x the fp16 rate), fp32 PSUM accumulate:

  sign(w) in {-1, 0, 1} is EXACT in e4m3, so all quantization error comes
  from x. e4m3(x) alone gives rel_l2 ~2.65e-2 (> the 2e-2 gate), so the K
  dimension is extended with a residual-correction batch:

    out = e4m3(x) @ s.T  +  e4m3(x - e4m3(x))[:, :CK] @ s[:, :CK].T  + b

  with CK = 1280. Measured rel_l2 = 1.62e-2. The correction batch reuses the
  same resident sign(w) SBUF tiles (no extra w traffic); only the residual
  x tiles (2.5 MiB/core) are extra DMA. PE work is (2048+1280)/2048 = 1.625x
  the pure-fp8 matmul but still ~2.4x below the fp16 floor.

  - the whole w^T shard [2048, 2048] e4m3 lives in SBUF (32 KiB/partition),
    loaded once;
  - x^T tiles stream through a multi-buffered pool;
  - bias is added during the PSUM->SBUF copyback on the vector engine and
    the result is stored as fp16 (halves output DMA; adds ~1e-4 rel error).
"""

from contextlib import ExitStack

import numpy as np

# Full problem shapes (hardcoded per the grading contract).
M, K, N = 8192, 2048, 4096
P_BATCH, P_FEAT = 4, 2  # 4 x 2 core grid
MC, NC = M // P_BATCH, N // P_FEAT  # 2048, 2048 per-core block
N_CORES = P_BATCH * P_FEAT
P = 128
CK = 1280  # corrected K prefix (residual batch)
KT0 = 512  # k-tile of the main (e4m3(x)) batch
KT1 = 256  # k-tile of the residual batch (1280 = 5 * 256)
TB = 512  # m/n tile width of the pre-blocked host layouts


def build_nc(mc: int = MC, k: int = K, nc_dim: int = NC, ck: int = CK, reps: int = 1):
    """Build + compile the per-core Bass module:
    out[mc, nc_dim] = x8^T.T @ w8 + r8^T.T @ w8[:ck] + bias (all fp8e4 inputs).

    reps > 1 repeats the whole computation (for slope-based benchmarking)."""
    import concourse.mybir as mybir
    import concourse.tile as tile
    from concourse import bacc
    from concourse.bass import ts
    from concourse.kernels.tile_matmul import (
        ShapeInfo,
        composable_matmul_tile_kernel,
    )

    ko = k // P  # 16 w subtiles resident in SBUF
    k_tiles0 = k // KT0  # 4
    ks0 = KT0 // P  # 4
    k_tiles1 = ck // KT1  # 5
    ks1 = KT1 // P  # 2
    m_tiles = mc // TB
    n_blocks = nc_dim // TB

    f8 = mybir.dt.float8e4

    nc = bacc.Bacc("TRN2", target_bir_lowering=False, debug=False)
    # Inputs arrive pre-blocked on the host (see _pack_blocks): each
    # [P, ks, TB] block is fully contiguous in DRAM, so every DMA has
    # multi-KiB-per-partition descriptor runs instead of strided ones.
    xt0 = nc.dram_tensor(
        "xt0", [m_tiles, k_tiles0, P, ks0, TB], f8, kind="ExternalInput"
    )
    xt1 = nc.dram_tensor(
        "xt1", [m_tiles, k_tiles1, P, ks1, TB], f8, kind="ExternalInput"
    )
    # w uses an n-block-major layout ([nb, p, ko, n]) so each n-block's
    # preload is contiguous per partition on BOTH sides.
    wt = nc.dram_tensor("wt", [n_blocks, P, ko, TB], f8, kind="ExternalInput")
    bias = nc.dram_tensor("bias", [nc_dim], mybir.dt.float32, kind="ExternalInput")
    out = nc.dram_tensor("out", [mc, nc_dim], mybir.dt.float16, kind="ExternalOutput")

    with tile.TileContext(nc) as tc, ExitStack() as ctx:
        # HAM warmup: the PE clock is gated to 1.2 GHz until ~3.4 us of
        # sustained activity. The first real matmuls can't start until their
        # operands arrive (~5 us of DMA ramp), so spend the idle window on
        # throwaway matmuls over a zeroed scratch tile — the cold-clock
        # penalty lands on them instead of the real work. The scratch SBUF
        # pool stays OPEN so its slot is never reused (a close would order
        # the w preload behind the dummy reads); only the PSUM bank is
        # returned before the real kernel needs all 8.
        warm_sb = ctx.enter_context(tc.tile_pool(name="warm_sb", bufs=1))
        scratch = warm_sb.tile([P, 512], mybir.dt.float16)
        nc.vector.memset(scratch[:], 0.0)
        with tc.tile_pool(name="warm_ps", bufs=1, space="PSUM") as wps_pool:
            ps = wps_pool.tile([P, 512], mybir.dt.float32)
            for _ in range(10):
                nc.tensor.matmul(
                    ps[:], scratch[:, :P], scratch[:], start=True, stop=True
                )

        const = ctx.enter_context(tc.tile_pool(name="const", bufs=1))
        kxm_pool = ctx.enter_context(
            tc.tile_pool(name="kxm", bufs=k_tiles0 + k_tiles1 + 1)
        )

        # Whole w^T shard resident in SBUF, n-block-major [p, nb, ko, n] with
        # cache[p, nb, o, j] = sign(w)^T[o*128 + p, nb*TB + j]. Preload runs on
        # the gpsimd (SWDGE) queue so the x-tile loads (HWDGE via nc.sync) are
        # not serialized behind it, in n-major order: the first output tile
        # consumes (n0, k0..k3), so all its chunks must land first.
        w_sb = const.tile([P, n_blocks, ko, TB], f8)
        for nb in range(n_blocks):
            if nb == 0:
                # first block split per k-tile so the first matmuls unblock
                # at k-tile granularity
                for kt in range(k_tiles0):
                    sl = slice(kt * ks0, (kt + 1) * ks0)
                    nc.gpsimd.dma_start(
                        out=w_sb[:, 0, sl, :], in_=wt.ap()[0, :, sl, :]
                    )
            else:
                nc.gpsimd.dma_start(out=w_sb[:, nb], in_=wt.ap()[nb])

        # Bias replicated across all 128 partitions so the copyback can add the
        # n-slice with a plain tensor_tensor add. One tiny [1, N] HBM read on
        # the otherwise-idle ACT HWDGE ring + an on-chip partition broadcast.
        bias_sb = const.tile([P, nc_dim], mybir.dt.float32)
        nc.scalar.dma_start(out=bias_sb[:1, :], in_=bias.ap()[None, :])
        nc.gpsimd.partition_broadcast(bias_sb[:], bias_sb[:1, :])

        # Custom kxm producer: one contiguous-block DMA per k-tile of x^T.
        def kxm_producer(nc_, md):
            if md.k_batch_idx == 0:
                t = kxm_pool.tile([P, ks0, md.m_tile], f8, tag="kxm")
                if md.m_tile_idx == 1 and n_blocks > 2:
                    # Ordering-only dep: m1's prefetch is not needed until
                    # late but otherwise jumps the shared DMA mover ahead of
                    # the w n1/n2 blocks. A tiny read of the w n2 region makes
                    # the real load schedule after that w block.
                    nc_.vector.tensor_copy(out=t[:1, :1, :2], in_=w_sb[:1, 2, :1, :2])
                nc_.sync.dma_start(out=t[:], in_=xt0.ap()[md.m_tile_idx, md.k_tile_idx])
            else:
                t = kxm_pool.tile([P, ks1, md.m_tile], f8, tag="kxm")
                nc_.sync.dma_start(out=t[:], in_=xt1.ap()[md.m_tile_idx, md.k_tile_idx])
            return t

        kxm_shape = ShapeInfo(pdims=((P, ko), (P, ck // P)), fdims=(mc,))

        def kxn_producer(nc_, md):
            if md.k_batch_idx == 0:
                return w_sb[:, md.n_tile_idx, ts(md.k_tile_idx, ks0), :]
            # Residual batch reuses the first ck columns of the resident w.
            return w_sb[:, md.n_tile_idx, ts(md.k_tile_idx, ks1), :]

        kxn_shape = ShapeInfo(pdims=((P, ko), (P, ck // P)), fdims=(nc_dim,))

        out_t = out.ap().rearrange("(o p) n -> p o n", p=P)

        def add_bias_store_reducer(nc_, psum, sbuf, md):
            # psum -> sbuf with the bias added (fp32 -> fp16), then store this
            # subtile immediately so stores overlap the remaining evictions.
            sz = md.n_subtile_slice_size
            nc_.vector.tensor_add(
                out=sbuf[:, :, :sz],
                in0=psum[:, :sz],
                in1=bias_sb[: psum.shape[0], md.n_subtile_slice],
            )
            po = md.m_tile_idx * md.m_subtiles + md.m_subtile_idx
            nc_.sync.dma_start(
                out=out_t[:, po : po + 1, md.n_subtile_slice], in_=sbuf[:, :, :sz]
            )

        for _ in range(reps):
            composable_matmul_tile_kernel(
                tc=tc,
                kxm_shape=kxm_shape,
                kxn_shape=kxn_shape,
                output_type=mybir.dt.float16,
                kxm_producer=kxm_producer,
                kxn_producer=kxn_producer,
                mxn_consumer=lambda nc_, tile_, md: None,
                mxn_subtile_reducer=add_bias_store_reducer,
                MAX_K_TILE_SIZE=KT0,
                psum_n_bufs=2,
            )

    nc.compile()
    return nc


def _f8(a: np.ndarray) -> np.ndarray:
    import ml_dtypes

    return a.astype(ml_dtypes.float8_e4m3)


def _pack_w_nblocks(a: np.ndarray, tb: int = TB) -> np.ndarray:
    """[N, K] (fp8) -> [N//tb, 128, K//128, tb] with
    block[nb, p, o, j] = a[nb*tb + j, o*128 + p]; per-partition-contiguous
    [ko, tb] planes -> multi-KiB DMA descriptor runs."""
    n, k = a.shape
    v = a.reshape(n // tb, tb, k // P, P)
    return np.ascontiguousarray(v.transpose(0, 3, 2, 1))


def _pack_blocks(a: np.ndarray, ktw: int, tb: int = TB) -> np.ndarray:
    """[F, K] (fp8) -> [F//tb, K//ktw, 128, ks, tb] DMA-contiguous blocks.

    block[ft, kt, p, s, j] = a[ft*tb + j, kt*ktw + s*128 + p], i.e. each
    [128, ks, tb] block is one fully-contiguous DMA source with K on the
    partition dim (a^T layout within the block)."""
    f, k = a.shape
    kts, ks = k // ktw, ktw // P
    v = a.reshape(f // tb, tb, kts, ks, P)
    return np.ascontiguousarray(v.transpose(0, 2, 4, 3, 1))


_NC_CACHE = None


def _get_nc():
    global _NC_CACHE
    if _NC_CACHE is None:
        _NC_CACHE = build_nc()
    return _NC_CACHE


def _pack_x_shard(xs: np.ndarray) -> tuple[np.ndarray, np.ndarray]:
    """f32 [MC, K] -> (xt0, xt1) fp8 DMA-blocked shards."""
    x8 = _f8(xs)
    r8 = _f8(xs - x8.astype(np.float32))
    return _pack_blocks(x8, KT0), _pack_blocks(r8[:, :CK], KT1)


def build_in_maps(x: np.ndarray, w: np.ndarray, b: np.ndarray) -> list[dict]:
    # Unique DMA-blocked shards (x per batch group, sign(w) per feature
    # group), packed in parallel (numpy releases the GIL on these copies).
    from concurrent.futures import ThreadPoolExecutor

    with ThreadPoolExecutor(max_workers=6) as pool:
        xt_f = [
            pool.submit(_pack_x_shard, x[bi * MC : (bi + 1) * MC])
            for bi in range(P_BATCH)
        ]
        wt_f = [
            pool.submit(
                lambda fi=fi: _pack_w_nblocks(_f8(np.sign(w[fi * NC : (fi + 1) * NC])))
            )
            for fi in range(P_FEAT)
        ]
        xt_shards = [f.result() for f in xt_f]
        wt_shards = [f.result() for f in wt_f]
    b_shards = [np.ascontiguousarray(b[fi * NC : (fi + 1) * NC]) for fi in range(P_FEAT)]

    in_maps = []
    for c in range(N_CORES):
        bi, fi = divmod(c, P_FEAT)
        in_maps.append(
            {
                "xt0": xt_shards[bi][0],
                "xt1": xt_shards[bi][1],
                "wt": wt_shards[fi],
                "bias": b_shards[fi],
            }
        )
    return in_maps


def kernel(x: np.ndarray, w: np.ndarray, b: np.ndarray) -> np.ndarray:
    from concourse.bass_utils import run_bass_kernel_spmd

    x = np.asarray(x, dtype=np.float32)
    w = np.asarray(w, dtype=np.float32)
    b = np.asarray(b, dtype=np.float32)

    in_maps = build_in_maps(x, w, b)

    nc = _get_nc()
    try:
        results = run_bass_kernel_spmd(
            nc, in_maps, core_ids=list(range(N_CORES))
        ).results
    except Exception:
        # One retry for transient runtime/relay failures.
        results = run_bass_kernel_spmd(
            nc, in_maps, core_ids=list(range(N_CORES))
        ).results

    out = np.empty((M, N), dtype=np.float32)
    for c in range(N_CORES):
        bi, fi = divmod(c, P_FEAT)
        out[bi * MC : (bi + 1) * MC, fi * NC : (fi + 1) * NC] = results[c][
            "out"
        ].astype(np.float32)
    return out


# revision 39
# speedup vs baseline: 2.1439x; 1.0708x over previous
"""BinaryLinear (straight-through sign(w)) kernel for Trainium2, 8 NeuronCores.

Computes out = x @ sign(w).T + b for
  x: [8192, 2048] f32, w: [4096, 2048] f32, b: [4096] f32 -> out [8192, 4096] f32.

Sharding: 4-way data parallel (batch) x 2-way tensor parallel (out_features).
Each core computes a [2048, 2048] block of the output.

Per-core device kernel — fp8e4 (e4m3) matmuls in DoubleRow perf mode (0.5
PE cycles per output row, 2x-4x the fp16 rate), fp32 PSUM accumulate:

  sign(w) in {-1, 0, 1} is EXACT in e4m3, so all quantization error comes
  from x. e4m3(x) alone gives rel_l2 ~2.65e-2 (> the 2e-2 gate), so the K
  dimension is extended with a residual-correction batch:

    out = e4m3(x) @ s.T  +  e4m3(x - e4m3(x))[:, :CK] @ s[:, :CK].T  + b

  with CK = 1280. Measured rel_l2 = 1.62e-2. The correction batch reuses the
  same resident sign(w) SBUF tiles (no extra w traffic); only the residual
  x tiles (2.5 MiB/core) are extra DMA. PE work is (2048+1280)/2048 = 1.625x
  the pure-fp8 matmul but still ~2.4x below the fp16 floor.

  - the whole w^T shard [2048, 2048] e4m3 lives in SBUF (32 KiB/partition),
    loaded once;
  - x^T tiles stream through a multi-buffered pool;
  - bias is added during the PSUM->SBUF copyback on the vector engine and
    the result is stored as fp16 (halves output DMA; adds ~1e-4 rel error).
"""

from contextlib import ExitStack

import numpy as np

# Full problem shapes (hardcoded per the grading contract).
M, K, N = 8192, 2048, 4096
P_BATCH, P_FEAT = 4, 2  # 4 x 2 core grid
MC, NC = M // P_BATCH, N // P_FEAT  # 2048, 2048 per-core block
N_CORES = P_BATCH * P_FEAT
P = 128
CK = 1280  # corrected K prefix (residual batch)
KT0 = 512  # k-tile of the main (e4m3(x)) batch
KT1 = 256  # k-tile of the residual batch (1280 = 5 * 256)
TB = 512  # m/n tile width of the pre-blocked host layouts


def build_nc(
    mc: int = MC,
    k: int = K,
    nc_dim: int = NC,
    ck: int = CK,
    reps: int = 1,
    store_rings: tuple = ("sync", "scalar"),
    evict_engines: tuple = ("vector", "vector", "vector", "gpsimd"),
    kxm_bufs: int | None = None,
    w_ring: str = "gpsimd",
    w_kt_snake: bool = True,
    temps_bufs: int = 12,
    direct_psum_store: bool = True,
    warmup_mm: int = 10,
    memset_w: int = 512,
):
    """Build + compile the per-core Bass module:
    out[mc, nc_dim] = x8^T.T @ w8 + r8^T.T @ w8[:ck] + bias (all fp8e4 inputs).

    reps > 1 repeats the whole computation (for slope-based benchmarking)."""
    import concourse.mybir as mybir
    import concourse.tile as tile
    from concourse import bacc
    from concourse.bass import ts
    from concourse.kernels.tile_matmul import (
        ShapeInfo,
        composable_matmul_tile_kernel,
    )

    ko = k // P  # 16 w subtiles resident in SBUF
    k_tiles0 = k // KT0  # 4
    ks0 = KT0 // P  # 4
    k_tiles1 = ck // KT1  # 5
    ks1 = KT1 // P  # 2
    m_tiles = mc // TB
    n_blocks = nc_dim // TB

    f8 = mybir.dt.float8e4

    # 64 KiB SWDGE descriptor carveout: the 16-chunk w preload emits 2048
    # descriptors; the default 16 KiB carveout (1024 descs) wraps twice, and
    # each wrap inserts a ring-drain sem wait that stalls an unrelated matmul.
    nc = bacc.Bacc(
        "TRN2",
        target_bir_lowering=False,
        debug=False,
        dynamic_dma_scratch_size=65536,
    )
    # Inputs arrive pre-blocked on the host (see _pack_blocks): each
    # [P, ks, TB] block is fully contiguous in DRAM, so every DMA has
    # multi-KiB-per-partition descriptor runs instead of strided ones.
    xt0 = nc.dram_tensor(
        "xt0", [m_tiles, k_tiles0, P, ks0, TB], f8, kind="ExternalInput"
    )
    xt1 = nc.dram_tensor(
        "xt1", [m_tiles, k_tiles1, P, ks1, TB], f8, kind="ExternalInput"
    )
    # w uses an n-block-major layout ([nb, p, ko, n]) so each n-block's
    # preload is contiguous per partition on BOTH sides.
    wt = nc.dram_tensor("wt", [n_blocks, P, ko, TB], f8, kind="ExternalInput")
    bias = nc.dram_tensor("bias", [nc_dim], mybir.dt.float32, kind="ExternalInput")
    out = nc.dram_tensor("out", [mc, nc_dim], mybir.dt.float16, kind="ExternalOutput")

    with tile.TileContext(nc) as tc, ExitStack() as ctx:
        # HAM warmup: the PE clock is gated to 1.2 GHz until ~3.4 us of
        # sustained activity. The first real matmuls can't start until their
        # operands arrive (~5 us of DMA ramp), so spend the idle window on
        # throwaway matmuls over a zeroed scratch tile — the cold-clock
        # penalty lands on them instead of the real work. The scratch SBUF
        # pool stays OPEN so its slot is never reused (a close would order
        # the w preload behind the dummy reads); only the PSUM bank is
        # returned before the real kernel needs all 8.
        warm_sb = ctx.enter_context(tc.tile_pool(name="warm_sb", bufs=1))
        scratch = warm_sb.tile([P, 512], mybir.dt.float16)
        nc.vector.memset(scratch[:, :memset_w], 0.0)
        with tc.tile_pool(name="warm_ps", bufs=1, space="PSUM") as wps_pool:
            ps = wps_pool.tile([P, 512], mybir.dt.float32)
            for _ in range(warmup_mm):
                nc.tensor.matmul(
                    ps[:], scratch[:, :P], scratch[:], start=True, stop=True
                )

        const = ctx.enter_context(tc.tile_pool(name="const", bufs=1))
        # Two full m-tiles worth of k-tile buffers (+1) so the next m-tile's
        # x loads all prefetch while the current one computes — otherwise the
        # PE stalls ~2.4 us at every m-tile boundary.
        kxm_pool = ctx.enter_context(
            tc.tile_pool(
                name="kxm",
                bufs=2 * (k_tiles0 + k_tiles1) + 1 if kxm_bufs is None else kxm_bufs,
            )
        )

        # Whole w^T shard resident in SBUF, n-block-major [p, nb, ko, n] with
        # cache[p, nb, o, j] = sign(w)^T[o*128 + p, nb*TB + j]. Preload runs on
        # the gpsimd (SWDGE) queue so the x-tile loads (HWDGE via nc.sync) are
        # not serialized behind it, in n-major order: the first output tile
        # consumes (n0, k0..k3), so all its chunks must land first.
        # Every n-block is split per k-tile: the first matmuls unblock at
        # k-tile granularity, and no w transfer occupies the shared DMA
        # engines for more than ~730 ns — a whole-block (2.9 us) transfer
        # head-of-line blocks the x-tile stream the PE is waiting on.
        if not direct_psum_store:
            # Bias first: it is tiny, but the partition broadcast runs on the
            # Pool engine — emitted after the w preload it queues behind ~17 us
            # of SWDGE descriptor generation, and every eviction (reading
            # bias_sb) then gates psum recycling on it.
            bias_sb = const.tile([P, nc_dim], mybir.dt.float32)
            nc.scalar.dma_start(out=bias_sb[:1, :], in_=bias.ap()[None, :])
            nc.gpsimd.partition_broadcast(bias_sb[:], bias_sb[:1, :])
        else:
            # Bias is added on the host; the device does pure copies, which
            # the Activation engine can also do — a third eviction engine,
            # and no bias broadcast on the congested Pool queue.
            bias_sb = None

        # Staging for the final group's merged store: its four subtile stores
        # would otherwise serialize on the shared HWDGE device (~630 ns each)
        # and leave four sequential DMA-sem drains in the kernel tail.
        tail_sb = const.tile([P, 4, TB], mybir.dt.float16)

        # The whole-shard w preload: SWDGE descriptor generation on Pool costs
        # ~8 ns/descriptor (128 per chunk) regardless of chunking — the per
        # k-tile chunks exist so each transfer occupies the shared DMA engines
        # for <800 ns and readers unblock at k-tile granularity.
        w_sb = const.tile([P, n_blocks, ko, TB], f8)
        for nb in range(n_blocks):
            ring = getattr(nc, w_ring)
            kts = range(k_tiles0)
            if w_kt_snake and nb % 2 == 1:
                # odd n-blocks are consumed k-reversed (snake order): load
                # their chunks high-k first so the first reader unblocks early
                kts = reversed(kts)
            for kt in kts:
                sl = slice(kt * ks0, (kt + 1) * ks0)
                ring.dma_start(out=w_sb[:, nb, sl, :], in_=wt.ap()[nb, :, sl, :])

        # Custom kxm producer: one contiguous-block DMA per k-tile of x^T.
        def kxm_producer(nc_, md):
            if md.k_batch_idx == 0:
                t = kxm_pool.tile([P, ks0, md.m_tile], f8, tag="kxm")
                if md.m_tile_idx == 1 and md.k_tile_idx == 0:
                    # Ordering-only dep: m1's prefetch burst (9 tiles, ~5 us
                    # of DMA) otherwise jumps the FIFO ahead of the w n1-n3
                    # chunks that the n-tile transitions are waiting on. A
                    # tiny read of the last n2 w chunk schedules the burst
                    # after w n0-n2 have transferred (n3 interleaves fine).
                    nc_.vector.tensor_copy(
                        out=t[:1, :1, :2], in_=w_sb[:1, 2, ko - 1 : ko, :2]
                    )
                nc_.sync.dma_start(out=t[:], in_=xt0.ap()[md.m_tile_idx, md.k_tile_idx])
            else:
                t = kxm_pool.tile([P, ks1, md.m_tile], f8, tag="kxm")
                nc_.sync.dma_start(out=t[:], in_=xt1.ap()[md.m_tile_idx, md.k_tile_idx])
            return t

        kxm_shape = ShapeInfo(pdims=((P, ko), (P, ck // P)), fdims=(mc,))

        def kxn_producer(nc_, md):
            if md.k_batch_idx == 0:
                return w_sb[:, md.n_tile_idx, ts(md.k_tile_idx, ks0), :]
            # Residual batch reuses the first ck columns of the resident w.
            return w_sb[:, md.n_tile_idx, ts(md.k_tile_idx, ks1), :]

        kxn_shape = ShapeInfo(pdims=((P, ko), (P, ck // P)), fdims=(nc_dim,))

        out_t = out.ap().rearrange("(o p) n -> p o n", p=P)

        def add_bias_store_reducer(nc_, psum, sbuf, md):
            sz = md.n_subtile_slice_size
            po = md.m_tile_idx * md.m_subtiles + md.m_subtile_idx
            ring = getattr(nc_, store_rings[md.m_subtile_idx % len(store_rings)])
            if direct_psum_store:
                # Host-side bias: the eviction is a pure fp32->fp16 copy, so
                # it can alternate between the Activation and DVE engines
                # (ACT has no tensor+tensor op, but Copy it can) — two
                # parallel eviction chains without touching the Pool queue.
                # The snake order ends at (m_tiles-1, n0): that group evicts
                # into one staging tile and ships as a single merged DMA.
                last_group = md.m_tile_idx == m_tiles - 1 and md.n_tile_idx == 0
                dst = tail_sb[:, md.m_subtile_idx : md.m_subtile_idx + 1, :sz] if last_group else sbuf[:, :, :sz]
                if md.m_subtile_idx % 2 == 0:
                    nc_.scalar.activation(
                        out=dst,
                        in_=psum[:, :sz],
                        func=mybir.ActivationFunctionType.Copy,
                    )
                else:
                    nc_.vector.tensor_copy(out=dst, in_=psum[:, :sz])
                if last_group:
                    if md.m_subtile_idx == md.m_subtiles - 1:
                        nc_.sync.dma_start(
                            out=out_t[
                                :,
                                po - (md.m_subtiles - 1) : po + 1,
                                md.n_subtile_slice,
                            ],
                            in_=tail_sb[:, :, :sz],
                        )
                    return
                ring.dma_start(
                    out=out_t[:, po : po + 1, md.n_subtile_slice], in_=sbuf[:, :, :sz]
                )
                return
            # psum -> sbuf with the bias added (fp32 -> fp16), then store this
            # subtile immediately so stores overlap the remaining evictions.
            # Evictions alternate DVE / Pool by subtile parity: the four
            # m-subtile psum groups of an (m, n) tile finish within ~430 ns of
            # each other, so a single engine serializes them (~600 ns apiece)
            # — that is the whole kernel tail after the last matmul.
            # The Pool sequencer spends the first ~17 us generating the w
            # preload's SWDGE descriptors; an eviction queued on it during
            # that window gates the next-next psum group (and thus the PE).
            # Keep the first m-tile's evictions on DVE only.
            if md.m_tile_idx == 0:
                eng = nc_.vector
            else:
                eng = getattr(nc_, evict_engines[md.m_subtile_idx % len(evict_engines)])
            eng.tensor_add(
                out=sbuf[:, :, :sz],
                in0=psum[:, :sz],
                in1=bias_sb[: psum.shape[0], md.n_subtile_slice],
            )
            ring.dma_start(
                out=out_t[:, po : po + 1, md.n_subtile_slice], in_=sbuf[:, :, :sz]
            )

        for _ in range(reps):
            composable_matmul_tile_kernel(
                tc=tc,
                kxm_shape=kxm_shape,
                kxn_shape=kxn_shape,
                output_type=mybir.dt.float16,
                kxm_producer=kxm_producer,
                kxn_producer=kxn_producer,
                mxn_consumer=lambda nc_, tile_, md: None,
                mxn_subtile_reducer=add_bias_store_reducer,
                MAX_K_TILE_SIZE=KT0,
                psum_n_bufs=2,
                # 12 output-staging tiles (1 KiB/partition each): the default 3
                # gates each eviction on the out-store DMA three evictions
                # back, which sits low-priority in the DMA queue — the PE then
                # stalls on psum recycling.
                temps_n_bufs=temps_bufs,
            )

    nc.compile()
    return nc


def _f8(a: np.ndarray) -> np.ndarray:
    import ml_dtypes

    return a.astype(ml_dtypes.float8_e4m3)


def _pack_w_nblocks(a: np.ndarray, tb: int = TB) -> np.ndarray:
    """[N, K] (fp8) -> [N//tb, 128, K//128, tb] with
    block[nb, p, o, j] = a[nb*tb + j, o*128 + p]; per-partition-contiguous
    [ko, tb] planes -> multi-KiB DMA descriptor runs."""
    n, k = a.shape
    v = a.reshape(n // tb, tb, k // P, P)
    return np.ascontiguousarray(v.transpose(0, 3, 2, 1))


def _pack_blocks(a: np.ndarray, ktw: int, tb: int = TB) -> np.ndarray:
    """[F, K] (fp8) -> [F//tb, K//ktw, 128, ks, tb] DMA-contiguous blocks.

    block[ft, kt, p, s, j] = a[ft*tb + j, kt*ktw + s*128 + p], i.e. each
    [128, ks, tb] block is one fully-contiguous DMA source with K on the
    partition dim (a^T layout within the block)."""
    f, k = a.shape
    kts, ks = k // ktw, ktw // P
    v = a.reshape(f // tb, tb, kts, ks, P)
    return np.ascontiguousarray(v.transpose(0, 2, 4, 3, 1))


_NC_CACHE = None


def _get_nc():
    global _NC_CACHE
    if _NC_CACHE is None:
        _NC_CACHE = build_nc()
    return _NC_CACHE


def _pack_x_shard(xs: np.ndarray) -> tuple[np.ndarray, np.ndarray]:
    """f32 [MC, K] -> (xt0, xt1) fp8 DMA-blocked shards."""
    x8 = _f8(xs)
    r8 = _f8(xs - x8.astype(np.float32))
    return _pack_blocks(x8, KT0), _pack_blocks(r8[:, :CK], KT1)


def build_in_maps(x: np.ndarray, w: np.ndarray, b: np.ndarray) -> list[dict]:
    # Unique DMA-blocked shards (x per batch group, sign(w) per feature
    # group), packed in parallel (numpy releases the GIL on these copies).
    from concurrent.futures import ThreadPoolExecutor

    with ThreadPoolExecutor(max_workers=6) as pool:
        xt_f = [
            pool.submit(_pack_x_shard, x[bi * MC : (bi + 1) * MC])
            for bi in range(P_BATCH)
        ]
        wt_f = [
            pool.submit(
                lambda fi=fi: _pack_w_nblocks(_f8(np.sign(w[fi * NC : (fi + 1) * NC])))
            )
            for fi in range(P_FEAT)
        ]
        xt_shards = [f.result() for f in xt_f]
        wt_shards = [f.result() for f in wt_f]
    b_shards = [np.ascontiguousarray(b[fi * NC : (fi + 1) * NC]) for fi in range(P_FEAT)]

    in_maps = []
    for c in range(N_CORES):
        bi, fi = divmod(c, P_FEAT)
        in_maps.append(
            {
                "xt0": xt_shards[bi][0],
                "xt1": xt_shards[bi][1],
                "wt": wt_shards[fi],
                "bias": b_shards[fi],
            }
        )
    return in_maps


def kernel(x: np.ndarray, w: np.ndarray, b: np.ndarray) -> np.ndarray:
    from concourse.bass_utils import run_bass_kernel_spmd

    x = np.asarray(x, dtype=np.float32)
    w = np.asarray(w, dtype=np.float32)
    b = np.asarray(b, dtype=np.float32)

    in_maps = build_in_maps(x, w, b)

    nc = _get_nc()
    try:
        results = run_bass_kernel_spmd(
            nc, in_maps, core_ids=list(range(N_CORES))
        ).results
    except Exception:
        # One retry for transient runtime/relay failures.
        results = run_bass_kernel_spmd(
            nc, in_maps, core_ids=list(range(N_CORES))
        ).results

    out = np.empty((M, N), dtype=np.float32)
    for c in range(N_CORES):
        bi, fi = divmod(c, P_FEAT)
        out[bi * MC : (bi + 1) * MC, fi * NC : (fi + 1) * NC] = results[c][
            "out"
        ].astype(np.float32)
    # The device stores raw psum (x @ sign(w).T); the bias lives on the host.
    out += b[None, :]
    return out


# revision 43
# speedup vs baseline: 2.2588x; 1.0536x over previous
"""BinaryLinear (straight-through sign(w)) kernel for Trainium2, 8 NeuronCores.

Computes out = x @ sign(w).T + b for
  x: [8192, 2048] f32, w: [4096, 2048] f32, b: [4096] f32 -> out [8192, 4096] f32.

Sharding: 4-way data parallel (batch) x 2-way tensor parallel (out_features).
Each core computes a [2048, 2048] block of the output.

Per-core device kernel — fp8e4 (e4m3) matmuls in DoubleRow perf mode (0.5
PE cycles per output row — 4x the fp16 MAC rate, confirmed on hardware),
fp32 PSUM accumulate:

  sign(w) in {-1, 0, 1} is EXACT in e4m3, so all quantization error comes
  from x. e4m3(x) alone gives rel_l2 ~2.65e-2 (> the 2e-2 gate), so the K
  dimension is extended with a residual-correction batch:

    out = e4m3(x) @ s.T  +  e4m3(x - e4m3(x))[:, :CK] @ s[:, :CK].T  + b

  with CK = 1280. Measured rel_l2 = 1.62e-2 (hardware-verified). The
  correction batch reuses the resident sign(w) SBUF tiles (no extra w
  traffic); only the residual x tiles (2.5 MiB/core) are extra DMA. PE work
  is (2048+1280)/2048 = 1.625x the pure-fp8 matmul but still ~2.4x below
  the fp16 floor.

Structure notes (all sim-traced against the TRN2 cost model):
  - whole w^T shard resident in SBUF (16 KiB/partition e4m3), preloaded on
    the gpsimd SWDGE ring in per-k-tile chunks (fine-grained reader deps,
    <800 ns head-of-line on the shared DMA engines);
  - x^T tiles stream through a 19-buffer pool (full next-m-tile prefetch);
  - swap_mm_args: psum holds [128 n-rows, 512 m] so the bias is a
    PER-PARTITION scalar — the psum->sbuf eviction fuses it on either the
    Activation engine (activation Copy with bias) or DVE (tensor_scalar
    add), two parallel eviction chains; output is stored n-major fp16 and
    the host transposes back;
  - the final group's four subtile stores merge into one DMA so the kernel
    tail is one eviction chain + one store + one sem drain.
"""

from contextlib import ExitStack

import numpy as np

# Full problem shapes (hardcoded per the grading contract).
M, K, N = 8192, 2048, 4096
P_BATCH, P_FEAT = 4, 2  # 4 x 2 core grid
MC, NC = M // P_BATCH, N // P_FEAT  # 2048, 2048 per-core block
N_CORES = P_BATCH * P_FEAT
P = 128
CK = 1280  # corrected K prefix (residual batch)
KT0 = 512  # k-tile of the main (e4m3(x)) batch
KT1 = 256  # k-tile of the residual batch (1280 = 5 * 256)
TB = 512  # m/n tile width of the pre-blocked host layouts


def build_nc(
    mc: int = MC,
    k: int = K,
    nc_dim: int = NC,
    ck: int = CK,
    reps: int = 1,
    store_rings: tuple = ("sync", "scalar"),
    kxm_bufs: int | None = None,
    w_kt_snake: bool = True,
    temps_bufs: int = 12,
    warmup_mm: int = 8,
):
    """Build + compile the per-core Bass module:
    out[nc_dim, mc] (n-major) = (x8 | r8) @ (w8 | w8[:, :ck]) + bias.

    reps > 1 repeats the whole computation (for slope-based benchmarking)."""
    import concourse.mybir as mybir
    import concourse.tile as tile
    from concourse import bacc
    from concourse.bass import ts
    from concourse.kernels.tile_matmul import (
        ShapeInfo,
        composable_matmul_tile_kernel,
    )

    ko = k // P  # 16 w subtiles resident in SBUF
    k_tiles0 = k // KT0  # 4
    ks0 = KT0 // P  # 4
    k_tiles1 = ck // KT1  # 5
    ks1 = KT1 // P  # 2
    m_tiles = mc // TB
    n_blocks = nc_dim // TB
    nbo = TB // P  # n-subtiles per n-block (4)

    f8 = mybir.dt.float8e4

    # 64 KiB SWDGE descriptor carveout: the 16-chunk w preload emits 2048
    # descriptors; the default 16 KiB carveout (1024 descs) wraps twice, and
    # each wrap inserts a ring-drain sem wait that stalls an unrelated matmul.
    nc = bacc.Bacc(
        "TRN2",
        target_bir_lowering=False,
        debug=False,
        dynamic_dma_scratch_size=65536,
    )
    # Inputs arrive pre-blocked on the host (see _pack_blocks): each
    # [P, ks, TB] block is fully contiguous in DRAM, so every DMA has
    # multi-KiB-per-partition descriptor runs instead of strided ones.
    xt0 = nc.dram_tensor(
        "xt0", [m_tiles, k_tiles0, P, ks0, TB], f8, kind="ExternalInput"
    )
    xt1 = nc.dram_tensor(
        "xt1", [m_tiles, k_tiles1, P, ks1, TB], f8, kind="ExternalInput"
    )
    # w uses an n-block-major layout ([nb, p, ko, n]) so each n-block's
    # preload is contiguous per partition on BOTH sides.
    wt = nc.dram_tensor("wt", [n_blocks, P, ko, TB], f8, kind="ExternalInput")
    # bias arrives pre-packed [128, n_subtiles]: bias_pk[p, o] = b[o*128 + p],
    # matching the psum partition layout (n on partitions under swap_mm_args).
    bias = nc.dram_tensor(
        "bias", [P, nc_dim // P], mybir.dt.float32, kind="ExternalInput"
    )
    # Output is stored n-major ([nc, mc]); the host transposes back.
    out = nc.dram_tensor("out", [nc_dim, mc], mybir.dt.float16, kind="ExternalOutput")

    with tile.TileContext(nc) as tc, ExitStack() as ctx:
        # HAM warmup: the PE clock is gated to 1.2 GHz until ~3.4 us of
        # sustained activity. The first real matmuls can't start until their
        # operands arrive (~3 us of DMA ramp), so spend the idle window on
        # throwaway matmuls over a zeroed scratch tile — the cold-clock
        # penalty lands on them instead of the real work. The scratch SBUF
        # pool stays OPEN so its slot is never reused (a close would order
        # the w preload behind the dummy reads); only the PSUM bank is
        # returned before the real kernel needs all 8.
        warm_sb = ctx.enter_context(tc.tile_pool(name="warm_sb", bufs=1))
        scratch = warm_sb.tile([P, 512], mybir.dt.float16)
        nc.vector.memset(scratch[:], 0.0)
        with tc.tile_pool(name="warm_ps", bufs=1, space="PSUM") as wps_pool:
            ps = wps_pool.tile([P, 512], mybir.dt.float32)
            for _ in range(warmup_mm):
                nc.tensor.matmul(
                    ps[:], scratch[:, :P], scratch[:], start=True, stop=True
                )

        const = ctx.enter_context(tc.tile_pool(name="const", bufs=1))
        # Two full m-tiles worth of k-tile buffers (+1) so the next m-tile's
        # x loads all prefetch while the current one computes — otherwise the
        # PE stalls ~2.4 us at every m-tile boundary.
        kxm_pool = ctx.enter_context(
            tc.tile_pool(
                name="kxm",
                bufs=2 * (k_tiles0 + k_tiles1) + 1 if kxm_bufs is None else kxm_bufs,
            )
        )

        # Packed bias [128, 16] on the ACT HWDGE ring (one 64 B/partition DMA).
        bias_sb = const.tile([P, nc_dim // P], mybir.dt.float32)
        nc.scalar.dma_start(out=bias_sb[:], in_=bias.ap())

        # Whole w^T shard resident in SBUF, n-block-major [p, nb, ko, n] with
        # cache[p, nb, o, j] = sign(w)^T[o*128 + p, nb*TB + j]. The preload
        # runs on the gpsimd SWDGE ring (descriptor generation there costs
        # ~8 ns/desc of Pool time but does not touch the HWDGE device the x
        # loads and out stores issue through). Per-k-tile chunks keep every
        # transfer <800 ns on the shared DMA engines and let readers unblock
        # at k-tile granularity.
        w_sb = const.tile([P, n_blocks, ko, TB], f8)
        for nb in range(n_blocks):
            kts = range(k_tiles0)
            if w_kt_snake and nb % 2 == 1:
                # odd n-blocks are consumed k-reversed (snake order): load
                # their chunks high-k first so the first reader unblocks early
                kts = reversed(kts)
            for kt in kts:
                sl = slice(kt * ks0, (kt + 1) * ks0)
                nc.gpsimd.dma_start(out=w_sb[:, nb, sl, :], in_=wt.ap()[nb, :, sl, :])

        # Staging for the final group's merged store: its four subtile stores
        # would otherwise serialize on the shared HWDGE device (~630 ns each)
        # and leave four sequential DMA-sem drains in the kernel tail.
        tail_sb = const.tile([P, nbo, TB], mybir.dt.float16)

        # Custom kxm producer: one contiguous-block DMA per k-tile of x^T.
        def kxm_producer(nc_, md):
            if md.k_batch_idx == 0:
                t = kxm_pool.tile([P, ks0, md.m_tile], f8, tag="kxm")
                if md.m_tile_idx == 1 and md.k_tile_idx == 0:
                    # Ordering-only dep: m1's prefetch burst (9 tiles, ~5 us
                    # of DMA) otherwise jumps the FIFO ahead of the w n1-n3
                    # chunks that the n-tile transitions are waiting on. A
                    # tiny read of the last n2 w chunk schedules the burst
                    # after w n0-n2 have transferred (n3 interleaves fine).
                    nc_.vector.tensor_copy(
                        out=t[:1, :1, :2], in_=w_sb[:1, 2, ko - 1 : ko, :2]
                    )
                nc_.sync.dma_start(out=t[:], in_=xt0.ap()[md.m_tile_idx, md.k_tile_idx])
            else:
                t = kxm_pool.tile([P, ks1, md.m_tile], f8, tag="kxm")
                nc_.sync.dma_start(out=t[:], in_=xt1.ap()[md.m_tile_idx, md.k_tile_idx])
            return t

        kxm_shape = ShapeInfo(pdims=((P, ko), (P, ck // P)), fdims=(mc,))

        def kxn_producer(nc_, md):
            if md.k_batch_idx == 0:
                return w_sb[:, md.n_tile_idx, ts(md.k_tile_idx, ks0), :]
            # Residual batch reuses the first ck columns of the resident w.
            return w_sb[:, md.n_tile_idx, ts(md.k_tile_idx, ks1), :]

        kxn_shape = ShapeInfo(pdims=((P, ko), (P, ck // P)), fdims=(nc_dim,))

        # n-major output view: out_t[p, o, m] = out[o*128 + p, m].
        out_t = out.ap().rearrange("(o p) m -> p o m", p=P)

        def bias_store_reducer(nc_, psum, sbuf, md):
            # swap_mm_args: psum is [128 n-rows, TB m-cols]; md.m_* fields
            # describe the psum FREE dim (m), md.m_subtile_idx indexes the
            # n-subtile within this n-tile. The bias is a per-partition
            # scalar here, so the eviction fuses it on either ACT
            # (activation-Copy with bias) or DVE (tensor_scalar add) — two
            # parallel eviction chains, and the Pool queue (busy generating
            # w descriptors for the first ~17 us) is never involved.
            o = md.n_tile_idx * nbo + md.m_subtile_idx
            m_lo = md.m_tile_idx * md.m_tile
            sz = md.m_tile
            last_group = md.m_tile_idx == m_tiles - 1 and md.n_tile_idx == 0
            dst = (
                tail_sb[:, md.m_subtile_idx : md.m_subtile_idx + 1, :sz]
                if last_group
                else sbuf[:, :, :sz]
            )
            if last_group and md.m_subtile_idx == 2:
                # Third eviction engine for the final group only — the Pool
                # queue has long finished the w preload by then.
                nc_.gpsimd.tensor_scalar_add(
                    out=dst, in0=psum[:, :sz], scalar1=bias_sb[:, o : o + 1]
                )
            elif md.m_subtile_idx % 2 == 0:
                nc_.scalar.activation(
                    out=dst,
                    in_=psum[:, :sz],
                    func=mybir.ActivationFunctionType.Identity,
                    scale=1.0,
                    bias=bias_sb[:, o : o + 1],
                )
            else:
                nc_.vector.tensor_scalar_add(
                    out=dst, in0=psum[:, :sz], scalar1=bias_sb[:, o : o + 1]
                )
            if last_group:
                # The snake order ends at (m_tiles-1, n0): that group evicts
                # into one staging tile and ships as two merged two-subtile
                # DMAs on separate rings — the first leaves right after the
                # second eviction, halving the exposed store tail.
                if md.m_subtile_idx == 1:
                    nc_.sync.dma_start(
                        out=out_t[:, 0:2, m_lo : m_lo + sz], in_=tail_sb[:, 0:2, :sz]
                    )
                elif md.m_subtile_idx == 3:
                    nc_.scalar.dma_start(
                        out=out_t[:, 2:4, m_lo : m_lo + sz], in_=tail_sb[:, 2:4, :sz]
                    )
                return
            ring = getattr(nc_, store_rings[md.m_subtile_idx % len(store_rings)])
            ring.dma_start(
                out=out_t[:, o : o + 1, m_lo : m_lo + sz], in_=dst
            )

        for _ in range(reps):
            composable_matmul_tile_kernel(
                tc=tc,
                kxm_shape=kxm_shape,
                kxn_shape=kxn_shape,
                output_type=mybir.dt.float16,
                kxm_producer=kxm_producer,
                kxn_producer=kxn_producer,
                mxn_consumer=lambda nc_, tile_, md: None,
                mxn_subtile_reducer=bias_store_reducer,
                MAX_K_TILE_SIZE=KT0,
                psum_n_bufs=2,
                # 12 output-staging tiles (1 KiB/partition each): the default
                # 3 gates each eviction on the out-store DMA three evictions
                # back, which sits low-priority in the DMA queue.
                temps_n_bufs=temps_bufs,
                swap_mm_args=True,
            )

    nc.compile()
    return nc


def _f8(a: np.ndarray) -> np.ndarray:
    import ml_dtypes

    return a.astype(ml_dtypes.float8_e4m3)


def _pack_w_nblocks(a: np.ndarray, tb: int = TB) -> np.ndarray:
    """[N, K] (fp8) -> [N//tb, 128, K//128, tb] with
    block[nb, p, o, j] = a[nb*tb + j, o*128 + p]; per-partition-contiguous
    [ko, tb] planes -> multi-KiB DMA descriptor runs."""
    n, k = a.shape
    v = a.reshape(n // tb, tb, k // P, P)
    return np.ascontiguousarray(v.transpose(0, 3, 2, 1))


def _pack_blocks(a: np.ndarray, ktw: int, tb: int = TB) -> np.ndarray:
    """[F, K] (fp8) -> [F//tb, K//ktw, 128, ks, tb] DMA-contiguous blocks.

    block[ft, kt, p, s, j] = a[ft*tb + j, kt*ktw + s*128 + p], i.e. each
    [128, ks, tb] block is one fully-contiguous DMA source with K on the
    partition dim (a^T layout within the block)."""
    f, k = a.shape
    kts, ks = k // ktw, ktw // P
    v = a.reshape(f // tb, tb, kts, ks, P)
    return np.ascontiguousarray(v.transpose(0, 2, 4, 3, 1))


_NC_CACHE = None


def _get_nc():
    global _NC_CACHE
    if _NC_CACHE is None:
        _NC_CACHE = build_nc()
    return _NC_CACHE


def _pack_x_shard(xs: np.ndarray) -> tuple[np.ndarray, np.ndarray]:
    """f32 [MC, K] -> (xt0, xt1) fp8 DMA-blocked shards."""
    x8 = _f8(xs)
    r8 = _f8(xs - x8.astype(np.float32))
    return _pack_blocks(x8, KT0), _pack_blocks(r8[:, :CK], KT1)


def build_in_maps(x: np.ndarray, w: np.ndarray, b: np.ndarray) -> list[dict]:
    # Unique DMA-blocked shards (x per batch group, sign(w) per feature
    # group), packed in parallel (numpy releases the GIL on these copies).
    from concurrent.futures import ThreadPoolExecutor

    with ThreadPoolExecutor(max_workers=6) as pool:
        xt_f = [
            pool.submit(_pack_x_shard, x[bi * MC : (bi + 1) * MC])
            for bi in range(P_BATCH)
        ]
        wt_f = [
            pool.submit(
                lambda fi=fi: _pack_w_nblocks(_f8(np.sign(w[fi * NC : (fi + 1) * NC])))
            )
            for fi in range(P_FEAT)
        ]
        xt_shards = [f.result() for f in xt_f]
        wt_shards = [f.result() for f in wt_f]
    b_shards = [
        np.ascontiguousarray(b[fi * NC : (fi + 1) * NC].reshape(NC // P, P).T)
        for fi in range(P_FEAT)
    ]

    in_maps = []
    for c in range(N_CORES):
        bi, fi = divmod(c, P_FEAT)
        in_maps.append(
            {
                "xt0": xt_shards[bi][0],
                "xt1": xt_shards[bi][1],
                "wt": wt_shards[fi],
                "bias": b_shards[fi],
            }
        )
    return in_maps


def kernel(x: np.ndarray, w: np.ndarray, b: np.ndarray) -> np.ndarray:
    from concourse.bass_utils import run_bass_kernel_spmd

    x = np.asarray(x, dtype=np.float32)
    w = np.asarray(w, dtype=np.float32)
    b = np.asarray(b, dtype=np.float32)

    in_maps = build_in_maps(x, w, b)

    nc = _get_nc()
    try:
        results = run_bass_kernel_spmd(
            nc, in_maps, core_ids=list(range(N_CORES))
        ).results
    except Exception:
        # One retry for transient runtime/relay failures.
        results = run_bass_kernel_spmd(
            nc, in_maps, core_ids=list(range(N_CORES))
        ).results

    out = np.empty((M, N), dtype=np.float32)
    for c in range(N_CORES):
        bi, fi = divmod(c, P_FEAT)
        # Device output is n-major [NC, MC] fp16; transpose back per block.
        out[bi * MC : (bi + 1) * MC, fi * NC : (fi + 1) * NC] = (
            results[c]["out"].astype(np.float32).T
        )
    return out


# revision 47
# speedup vs baseline: 2.2728x; 1.0062x over previous
"""BinaryLinear (straight-through sign(w)) kernel for Trainium2, 8 NeuronCores.

Computes out = x @ sign(w).T + b for
  x: [8192, 2048] f32, w: [4096, 2048] f32, b: [4096] f32 -> out [8192, 4096] f32.

Sharding: 4-way data parallel (batch) x 2-way tensor parallel (out_features).
Each core computes a [2048, 2048] block of the output.

Per-core device kernel — fp8e4 (e4m3) matmuls in DoubleRow perf mode (0.5
PE cycles per output row — 4x the fp16 MAC rate, confirmed on hardware),
fp32 PSUM accumulate:

  sign(w) in {-1, 0, 1} is EXACT in e4m3, so all quantization error comes
  from x. e4m3(x) alone gives rel_l2 ~2.65e-2 (> the 2e-2 gate), so the K
  dimension is extended with a residual-correction batch:

    out = e4m3(x) @ s.T  +  e4m3(x - e4m3(x))[:, :CK] @ s[:, :CK].T  + b

  with CK = 1280. Measured rel_l2 = 1.62e-2 (hardware-verified). The
  correction batch reuses the resident sign(w) SBUF tiles (no extra w
  traffic); only the residual x tiles (2.5 MiB/core) are extra DMA. PE work
  is (2048+1280)/2048 = 1.625x the pure-fp8 matmul but still ~2.4x below
  the fp16 floor.

Structure notes (all sim-traced against the TRN2 cost model):
  - whole w^T shard resident in SBUF (16 KiB/partition e4m3), preloaded on
    the gpsimd SWDGE ring in per-k-tile chunks (fine-grained reader deps,
    <800 ns head-of-line on the shared DMA engines);
  - x^T tiles stream through a 19-buffer pool (full next-m-tile prefetch);
  - swap_mm_args: psum holds [128 n-rows, 512 m] so the bias is a
    PER-PARTITION scalar — the psum->sbuf eviction fuses it on either the
    Activation engine (activation Copy with bias) or DVE (tensor_scalar
    add), two parallel eviction chains; output is stored n-major fp16 and
    the host transposes back;
  - the final group's four subtile stores merge into one DMA so the kernel
    tail is one eviction chain + one store + one sem drain.
"""

from contextlib import ExitStack

import numpy as np

# Full problem shapes (hardcoded per the grading contract).
M, K, N = 8192, 2048, 4096
P_BATCH, P_FEAT = 4, 2  # 4 x 2 core grid
MC, NC = M // P_BATCH, N // P_FEAT  # 2048, 2048 per-core block
N_CORES = P_BATCH * P_FEAT
P = 128
CK = 1280  # corrected K prefix (residual batch)
KT0 = 512  # k-tile of the main (e4m3(x)) batch
KT1 = 256  # k-tile of the residual batch (1280 = 5 * 256)
TB = 512  # m/n tile width of the pre-blocked host layouts


def build_nc(
    mc: int = MC,
    k: int = K,
    nc_dim: int = NC,
    ck: int = CK,
    reps: int = 1,
    store_rings: tuple = ("sync", "scalar"),
    kxm_bufs: int | None = None,
    temps_bufs: int = 12,
    warmup_mm: int = 8,
    defer_w: bool = True,
):
    """Build + compile the per-core Bass module:
    out[nc_dim, mc] (n-major) = (x8 | r8) @ (w8 | w8[:, :ck]) + bias.

    reps > 1 repeats the whole computation (for slope-based benchmarking)."""
    import concourse.mybir as mybir
    import concourse.tile as tile
    from concourse import bacc
    from concourse.bass import ts
    from concourse.kernels.tile_matmul import (
        ShapeInfo,
        composable_matmul_tile_kernel,
    )

    ko = k // P  # 16 w subtiles resident in SBUF
    k_tiles0 = k // KT0  # 4
    ks0 = KT0 // P  # 4
    k_tiles1 = ck // KT1  # 5
    ks1 = KT1 // P  # 2
    m_tiles = mc // TB
    n_blocks = nc_dim // TB
    nbo = TB // P  # n-subtiles per n-block (4)

    f8 = mybir.dt.float8e4

    # 64 KiB SWDGE descriptor carveout: the 16-chunk w preload emits 2048
    # descriptors; the default 16 KiB carveout (1024 descs) wraps twice, and
    # each wrap inserts a ring-drain sem wait that stalls an unrelated matmul.
    nc = bacc.Bacc(
        "TRN2",
        target_bir_lowering=False,
        debug=False,
        dynamic_dma_scratch_size=65536,
    )
    # Inputs arrive pre-blocked on the host (see _pack_blocks): each
    # [P, ks, TB] block is fully contiguous in DRAM, so every DMA has
    # multi-KiB-per-partition descriptor runs instead of strided ones.
    xt0 = nc.dram_tensor(
        "xt0", [m_tiles, k_tiles0, P, ks0, TB], f8, kind="ExternalInput"
    )
    xt1 = nc.dram_tensor(
        "xt1", [m_tiles, k_tiles1, P, ks1, TB], f8, kind="ExternalInput"
    )
    # w uses an n-block-major layout ([nb, p, ko, n]) so each n-block's
    # preload is contiguous per partition on BOTH sides.
    wt = nc.dram_tensor("wt", [n_blocks, P, ko, TB], f8, kind="ExternalInput")
    # bias arrives pre-packed [128, n_subtiles]: bias_pk[p, o] = b[o*128 + p],
    # matching the psum partition layout (n on partitions under swap_mm_args).
    bias = nc.dram_tensor(
        "bias", [P, nc_dim // P], mybir.dt.float32, kind="ExternalInput"
    )
    # Output is stored n-major ([nc, mc]); the host transposes back.
    out = nc.dram_tensor("out", [nc_dim, mc], mybir.dt.float16, kind="ExternalOutput")

    with tile.TileContext(nc) as tc, ExitStack() as ctx:
        # HAM warmup: the PE clock is gated to 1.2 GHz until ~3.4 us of
        # sustained activity. The first real matmuls can't start until their
        # operands arrive (~3 us of DMA ramp), so spend the idle window on
        # throwaway matmuls over a zeroed scratch tile — the cold-clock
        # penalty lands on them instead of the real work. The scratch SBUF
        # pool stays OPEN so its slot is never reused (a close would order
        # the w preload behind the dummy reads); only the PSUM bank is
        # returned before the real kernel needs all 8.
        warm_sb = ctx.enter_context(tc.tile_pool(name="warm_sb", bufs=1))
        scratch = warm_sb.tile([P, 512], mybir.dt.float16)
        nc.vector.memset(scratch[:], 0.0)
        with tc.tile_pool(name="warm_ps", bufs=1, space="PSUM") as wps_pool:
            ps = wps_pool.tile([P, 512], mybir.dt.float32)
            for _ in range(warmup_mm):
                nc.tensor.matmul(
                    ps[:], scratch[:, :P], scratch[:], start=True, stop=True
                )

        const = ctx.enter_context(tc.tile_pool(name="const", bufs=1))
        # Two full m-tiles worth of k-tile buffers (+1) so the next m-tile's
        # x loads all prefetch while the current one computes — otherwise the
        # PE stalls ~2.4 us at every m-tile boundary.
        kxm_pool = ctx.enter_context(
            tc.tile_pool(
                name="kxm",
                bufs=2 * (k_tiles0 + k_tiles1) + 1 if kxm_bufs is None else kxm_bufs,
            )
        )

        # Packed bias [128, 16] on the ACT HWDGE ring (one 64 B/partition DMA).
        bias_sb = const.tile([P, nc_dim // P], mybir.dt.float32)
        nc.scalar.dma_start(out=bias_sb[:], in_=bias.ap())

        # Pre-issue m0's last x8 k-tile so a tile handle exists for the
        # FIFO-ordering marker below (its DMA is first on the sync ring; the
        # producer returns this tile for (m0, batch0, kt3)).
        m0k3 = kxm_pool.tile([P, ks0, TB], f8, tag="kxm")
        nc.sync.dma_start(out=m0k3[:], in_=xt0.ap()[0, k_tiles0 - 1])

        # Whole w^T shard resident in SBUF, n-block-major [p, nb, ko, n] with
        # cache[p, nb, o, j] = sign(w)^T[o*128 + p, nb*TB + j]. The preload
        # runs on the gpsimd SWDGE ring (descriptor generation there costs
        # ~8 ns/desc of Pool time but does not touch the HWDGE device the x
        # loads and out stores issue through). Per-k-tile chunks keep every
        # transfer <800 ns on the shared DMA engines and let readers unblock
        # at k-tile granularity.
        #
        # Chunk order = consumption order: even n-blocks run k-forward, odd
        # ones k-reversed under the snake — and the reversed k-loop touches
        # the residual batch first, whose w lives in kt2..kt0; kt3 is only
        # needed once the reversed main batch reaches its last k-tile.
        w_sb = const.tile([P, n_blocks, ko, TB], f8)
        mark = const.tile([P, 2], mybir.dt.float16)

        def _preload_block(nb):
            kts = [2, 1, 0, 3] if nb % 2 == 1 else range(k_tiles0)
            for kt in kts:
                sl = slice(kt * ks0, (kt + 1) * ks0)
                nc.gpsimd.dma_start(out=w_sb[:, nb, sl, :], in_=wt.ap()[nb, :, sl, :])

        _preload_block(0)
        if defer_w:
            # Ordering-only dep: the n1-n3 chunk requests otherwise enter the
            # shared DMA FIFO ahead of m0's later x-tile requests and delay
            # the first psum group ~3 us. A tiny Pool read of the pre-issued
            # m0 x8-k3 tile parks the Pool queue until the whole m0 x stream
            # is in the FIFO; n1+ chunks then transfer right behind it.
            nc.gpsimd.tensor_copy(out=mark[:1, :2], in_=m0k3[:1, 0, :2])
        for nb in range(1, n_blocks):
            _preload_block(nb)

        # Staging for the final group's merged store: its four subtile stores
        # would otherwise serialize on the shared HWDGE device (~630 ns each)
        # and leave four sequential DMA-sem drains in the kernel tail.
        tail_sb = const.tile([P, nbo, TB], mybir.dt.float16)

        # Custom kxm producer: one contiguous-block DMA per k-tile of x^T.
        def kxm_producer(nc_, md):
            if md.k_batch_idx == 0:
                if md.m_tile_idx == 0 and md.k_tile_idx == k_tiles0 - 1:
                    return m0k3  # pre-issued above (FIFO marker tile)
                t = kxm_pool.tile([P, ks0, md.m_tile], f8, tag="kxm")
                if md.m_tile_idx == 1 and md.k_tile_idx == 0:
                    # Ordering-only dep: m1's prefetch burst (9 tiles, ~5 us
                    # of DMA) otherwise jumps the FIFO ahead of the w n1-n3
                    # chunks that the n-tile transitions are waiting on. A
                    # tiny read of the last n2 w chunk schedules the burst
                    # after w n0-n2 have transferred (n3 interleaves fine).
                    nc_.vector.tensor_copy(
                        out=t[:1, :1, :2], in_=w_sb[:1, 2, ko - 1 : ko, :2]
                    )
                nc_.sync.dma_start(out=t[:], in_=xt0.ap()[md.m_tile_idx, md.k_tile_idx])
            else:
                t = kxm_pool.tile([P, ks1, md.m_tile], f8, tag="kxm")
                nc_.sync.dma_start(out=t[:], in_=xt1.ap()[md.m_tile_idx, md.k_tile_idx])
            return t

        kxm_shape = ShapeInfo(pdims=((P, ko), (P, ck // P)), fdims=(mc,))

        def kxn_producer(nc_, md):
            if md.k_batch_idx == 0:
                return w_sb[:, md.n_tile_idx, ts(md.k_tile_idx, ks0), :]
            # Residual batch reuses the first ck columns of the resident w.
            return w_sb[:, md.n_tile_idx, ts(md.k_tile_idx, ks1), :]

        kxn_shape = ShapeInfo(pdims=((P, ko), (P, ck // P)), fdims=(nc_dim,))

        # n-major output view: out_t[p, o, m] = out[o*128 + p, m].
        out_t = out.ap().rearrange("(o p) m -> p o m", p=P)

        def bias_store_reducer(nc_, psum, sbuf, md):
            # swap_mm_args: psum is [128 n-rows, TB m-cols]; md.m_* fields
            # describe the psum FREE dim (m), md.m_subtile_idx indexes the
            # n-subtile within this n-tile. The bias is a per-partition
            # scalar here, so the eviction fuses it on either ACT
            # (activation-Copy with bias) or DVE (tensor_scalar add) — two
            # parallel eviction chains, and the Pool queue (busy generating
            # w descriptors for the first ~17 us) is never involved.
            o = md.n_tile_idx * nbo + md.m_subtile_idx
            m_lo = md.m_tile_idx * md.m_tile
            sz = md.m_tile
            last_group = md.m_tile_idx == m_tiles - 1 and md.n_tile_idx == 0
            dst = (
                tail_sb[:, md.m_subtile_idx : md.m_subtile_idx + 1, :sz]
                if last_group
                else sbuf[:, :, :sz]
            )
            # NOTE: gpsimd (Pool) cannot read PSUM with TensorScalarPtr — the
            # BIR verifier rejects it (the cost model does not model this).
            if md.m_subtile_idx % 2 == 0:
                nc_.scalar.activation(
                    out=dst,
                    in_=psum[:, :sz],
                    func=mybir.ActivationFunctionType.Identity,
                    scale=1.0,
                    bias=bias_sb[:, o : o + 1],
                )
            else:
                nc_.vector.tensor_scalar_add(
                    out=dst, in0=psum[:, :sz], scalar1=bias_sb[:, o : o + 1]
                )
            if last_group:
                # The snake order ends at (m_tiles-1, n0): that group evicts
                # into one staging tile and ships as two merged two-subtile
                # DMAs on separate rings — the first leaves right after the
                # second eviction, halving the exposed store tail.
                if md.m_subtile_idx == 1:
                    nc_.sync.dma_start(
                        out=out_t[:, 0:2, m_lo : m_lo + sz], in_=tail_sb[:, 0:2, :sz]
                    )
                elif md.m_subtile_idx == 3:
                    nc_.scalar.dma_start(
                        out=out_t[:, 2:4, m_lo : m_lo + sz], in_=tail_sb[:, 2:4, :sz]
                    )
                return
            ring = getattr(nc_, store_rings[md.m_subtile_idx % len(store_rings)])
            ring.dma_start(
                out=out_t[:, o : o + 1, m_lo : m_lo + sz], in_=dst
            )

        for _ in range(reps):
            composable_matmul_tile_kernel(
                tc=tc,
                kxm_shape=kxm_shape,
                kxn_shape=kxn_shape,
                output_type=mybir.dt.float16,
                kxm_producer=kxm_producer,
                kxn_producer=kxn_producer,
                mxn_consumer=lambda nc_, tile_, md: None,
                mxn_subtile_reducer=bias_store_reducer,
                MAX_K_TILE_SIZE=KT0,
                psum_n_bufs=2,
                # 12 output-staging tiles (1 KiB/partition each): the default
                # 3 gates each eviction on the out-store DMA three evictions
                # back, which sits low-priority in the DMA queue.
                temps_n_bufs=temps_bufs,
                swap_mm_args=True,
            )

    nc.compile()
    return nc


def _f8(a: np.ndarray) -> np.ndarray:
    import ml_dtypes

    return a.astype(ml_dtypes.float8_e4m3)


def _pack_w_nblocks(a: np.ndarray, tb: int = TB) -> np.ndarray:
    """[N, K] (fp8) -> [N//tb, 128, K//128, tb] with
    block[nb, p, o, j] = a[nb*tb + j, o*128 + p]; per-partition-contiguous
    [ko, tb] planes -> multi-KiB DMA descriptor runs."""
    n, k = a.shape
    v = a.reshape(n // tb, tb, k // P, P)
    return np.ascontiguousarray(v.transpose(0, 3, 2, 1))


def _pack_blocks(a: np.ndarray, ktw: int, tb: int = TB) -> np.ndarray:
    """[F, K] (fp8) -> [F//tb, K//ktw, 128, ks, tb] DMA-contiguous blocks.

    block[ft, kt, p, s, j] = a[ft*tb + j, kt*ktw + s*128 + p], i.e. each
    [128, ks, tb] block is one fully-contiguous DMA source with K on the
    partition dim (a^T layout within the block)."""
    f, k = a.shape
    kts, ks = k // ktw, ktw // P
    v = a.reshape(f // tb, tb, kts, ks, P)
    return np.ascontiguousarray(v.transpose(0, 2, 4, 3, 1))


_NC_CACHE = None


def _get_nc():
    global _NC_CACHE
    if _NC_CACHE is None:
        _NC_CACHE = build_nc()
    return _NC_CACHE


def _pack_x_shard(xs: np.ndarray) -> tuple[np.ndarray, np.ndarray]:
    """f32 [MC, K] -> (xt0, xt1) fp8 DMA-blocked shards."""
    x8 = _f8(xs)
    r8 = _f8(xs - x8.astype(np.float32))
    return _pack_blocks(x8, KT0), _pack_blocks(r8[:, :CK], KT1)


def build_in_maps(x: np.ndarray, w: np.ndarray, b: np.ndarray) -> list[dict]:
    # Unique DMA-blocked shards (x per batch group, sign(w) per feature
    # group), packed in parallel (numpy releases the GIL on these copies).
    from concurrent.futures import ThreadPoolExecutor

    with ThreadPoolExecutor(max_workers=6) as pool:
        xt_f = [
            pool.submit(_pack_x_shard, x[bi * MC : (bi + 1) * MC])
            for bi in range(P_BATCH)
        ]
        wt_f = [
            pool.submit(
                lambda fi=fi: _pack_w_nblocks(_f8(np.sign(w[fi * NC : (fi + 1) * NC])))
            )
            for fi in range(P_FEAT)
        ]
        xt_shards = [f.result() for f in xt_f]
        wt_shards = [f.result() for f in wt_f]
    b_shards = [
        np.ascontiguousarray(b[fi * NC : (fi + 1) * NC].reshape(NC // P, P).T)
        for fi in range(P_FEAT)
    ]

    in_maps = []
    for c in range(N_CORES):
        bi, fi = divmod(c, P_FEAT)
        in_maps.append(
            {
                "xt0": xt_shards[bi][0],
                "xt1": xt_shards[bi][1],
                "wt": wt_shards[fi],
                "bias": b_shards[fi],
            }
        )
    return in_maps


def kernel(x: np.ndarray, w: np.ndarray, b: np.ndarray) -> np.ndarray:
    from concourse.bass_utils import run_bass_kernel_spmd

    x = np.asarray(x, dtype=np.float32)
    w = np.asarray(w, dtype=np.float32)
    b = np.asarray(b, dtype=np.float32)

    in_maps = build_in_maps(x, w, b)

    nc = _get_nc()
    try:
        results = run_bass_kernel_spmd(
            nc, in_maps, core_ids=list(range(N_CORES))
        ).results
    except Exception:
        # One retry for transient runtime/relay failures.
        results = run_bass_kernel_spmd(
            nc, in_maps, core_ids=list(range(N_CORES))
        ).results

    out = np.empty((M, N), dtype=np.float32)
    for c in range(N_CORES):
        bi, fi = divmod(c, P_FEAT)
        # Device output is n-major [NC, MC] fp16; transpose back per block.
        out[bi * MC : (bi + 1) * MC, fi * NC : (fi + 1) * NC] = (
            results[c]["out"].astype(np.float32).T
        )
    return out
